# revision 27
# baseline (speedup 1.0000x reference)
"""Trainium2 Bass kernel for one backward-Euler implicit 1D diffusion step
(Thomas tridiagonal solve) on an 8,388,608-point grid, distributed over 8
NeuronCores.

Math: the tridiagonal system (I - dt*D*Lap) x = d has constant coefficients
a = c = -r, b = 1+2r with r = D*dt/dx^2 = 0.1 (Dirichlet rows at the two
ends).  The matrix is strongly diagonally dominant, so rows of its inverse
decay geometrically (ratio q ~ 0.084 per step).  To well below the 2e-2
tolerance the solve is therefore a 9-tap symmetric FIR convolution of the
RHS (truncation tail ~8e-6 relative), except within ~30 points of the two
global boundaries, which are recomputed exactly on the host (the trivially
small "reduced interface system" of the domain-decomposition approach).

Device implementation (overlap-save, bf16): each core owns a contiguous
1,048,576-point chunk.  The host quantizes the RHS to bf16 (quantization
~1e-3 abs, well inside tolerance) and shards it into overlapping 128-point
windows with stride S = 128 - 2K = 120: SBUF tile R[p, f] = d[S*f + p - K].
One banded 128x128 bf16 weight matrix W (W[p, i] = w[p-K-i]) computes all
S=120 valid outputs of every window in a single TensorE matmul pass:
out[i, f] = x[S*f + i] for i < S.  bf16 halves HBM traffic vs fp32:
~2.24 MB in + ~2.10 MB out per core = ~12.2 us at the 358 GB/s per-core
HBM roofline.  The first input chunk is small so the matmul stream starts
early, and outputs are flushed in 7 chunks round-robin over all three DMA
paths (SWDGE + both HWDGE rings) so stores drain behind the compute and
the post-compute tail is one tiny 35-column store.
"""

from contextlib import ExitStack

import ml_dtypes
import numpy as np

import concourse.bacc as bacc
import concourse.mybir as mybir
import concourse.tile as tile

N = 8_388_608
NCORES = 8
P = 128
PER_CORE = N // NCORES            # 1,048,576
K = 2                             # FIR radius (5 taps; tail ~1e-3, tol is 2e-2)
S = P - 2 * K                     # 124 valid outputs per window
NCOLS = -(-PER_CORE // S)         # 8,457 windows per core
NF = 1024                         # PSUM tile free dim (2 banks; matmul ISA max 512)
FIX = 512                         # host boundary fix-up length

BF16 = ml_dtypes.bfloat16

# matmul group sizes along the window axis: two small groups up front so the
# first output chunk is produced (and its store issued) as early as possible
GROUP_SIZES = [512, 512] + [NF] * 7 + [NCOLS - 1024 - 7 * NF]

# input chunk edges (cols, group-aligned): small first chunk so compute
# starts early.  HARD CONSTRAINT: the Tile scheduler has only 8 HWDGE DMA
# semaphore lanes (sync+scalar combined); exceeding 8 HWDGE DMAs forces lane
# recycling whose reset-waits couple DMA issue to compute progress and stall
# a whole ring.  Here: HWDGE = W + 5 loads + 2 stores = 8 exactly; SWDGE
# (own lane pool) carries the 3 mid-stream stores.
IN_EDGES = [0, 512, 2048, 5120, 7168, NCOLS]
# output store edges (cols, at group boundaries): last modest for short drain
OUT_EDGES = [0, 2048, 4096, 6144, 7168, NCOLS]

# stash of the last BassKernelResults for test harnesses
LAST_RESULTS = None
_NC_CACHE = None


def _coeffs(dt):
    """fp32 tridiagonal coefficients exactly as the reference computes them."""
    dtf = np.float32(dt)
    r = np.float32(np.float32(1e-9) * dtf) / np.float32(1e-4 * 1e-4)
    a = np.float32(-r)
    b = np.float32(np.float32(1.0) + np.float32(2.0) * r)
    c = np.float32(-r)
    return r, a, b, c


def _fir_taps(a, b, c):
    """Centered row of inv(tridiag(a,b,c)) in fp64: the 2K+1 FIR taps."""
    M = 4096
    af, bf, cf = float(a), float(b), float(c)
    d = np.zeros(M)
    d[M // 2] = 1.0
    cp = np.empty(M)
    dp = np.empty(M)
    cp[0] = cf / bf
    dp[0] = d[0] / bf
    for i in range(1, M):
        den = bf - af * cp[i - 1]
        cp[i] = cf / den
        dp[i] = (d[i] - af * dp[i - 1]) / den
    x = np.empty(M)
    x[-1] = dp[-1]
    for i in range(M - 2, -1, -1):
        x[i] = dp[i] - cp[i] * x[i + 1]
    return x[M // 2 - K : M // 2 + K + 1]


def _weight_mat(w):
    """Banded lhsT weight matrix: out[i,f] = sum_p W[p,i] R[p,f]."""
    W = np.zeros((P, P), dtype=np.float64)
    for p in range(P):
        for i in range(S):
            j = p - K - i
            if -K <= j <= K:
                W[p, i] = w[j + K]
    return W.astype(BF16)


def _build_device_program():
    nc = bacc.Bacc("TRN2", debug=False)
    R = nc.dram_tensor("r_in", [P, NCOLS], mybir.dt.bfloat16, kind="ExternalInput")
    WT = nc.dram_tensor("w_in", [P, P], mybir.dt.bfloat16, kind="ExternalInput")
    # full 128 output rows: stores of non-128-partition tiles fall off the
    # fast 16-queue spray descriptor path and run ~4x slower (measured).
    # Rows S..127 are zeros (W's padding rows); the host discards them.
    X = nc.dram_tensor("x_out", [P, NCOLS], mybir.dt.bfloat16, kind="ExternalOutput")

    with tile.TileContext(nc) as tc, ExitStack() as ctx:
        wpool = ctx.enter_context(tc.tile_pool(name="w", bufs=1))
        epool = ctx.enter_context(tc.tile_pool(name="e", bufs=1))
        psum = ctx.enter_context(tc.tile_pool(name="ps", bufs=4, space="PSUM"))
        opool = ctx.enter_context(tc.tile_pool(name="o", bufs=1))

        # loads split across BOTH HWDGE rings (one ring sustains only ~200
        # GB/s; two reach the ~358 GB/s HBM roofline).  The first chunks ride
        # sync because scalar's hoisted ACT_TABLE_LOAD delays its first DMA
        # by ~1.3us.  Weights first (tiny, and the matmuls need them).
        w_t = wpool.tile([P, P], mybir.dt.bfloat16)
        nc.sync.dma_start(w_t[:], WT[:, :])

        e_t = epool.tile([P, NCOLS], mybir.dt.bfloat16)
        in_engines = [nc.sync, nc.scalar, nc.sync, nc.scalar, nc.sync]
        for eng, (lo, hi) in zip(in_engines, zip(IN_EDGES, IN_EDGES[1:])):
            eng.dma_start(e_t[:, lo:hi], R[:, lo:hi])

        # one big output SBUF tile (valid rows 0..S), flushed in chunks
        # round-robin over all three DMA paths right behind the compute
        o_t = opool.tile([P, NCOLS], mybir.dt.bfloat16)
        out_engines = [nc.gpsimd, nc.gpsimd, nc.gpsimd, nc.scalar, nc.sync]

        oi = 0
        c0 = 0
        MM = 512                  # ISA matmul free-dim limit per instruction
        for g, gw in enumerate(GROUP_SIZES):
            ps = psum.tile([P, NF], mybir.dt.float32, tag="ps")
            # fill the 2-bank PSUM tile with 512-col matmuls
            for m0 in range(0, gw, MM):
                mw = min(MM, gw - m0)
                nc.tensor.matmul(
                    ps[:, m0 : m0 + mw],
                    w_t[:],
                    e_t[:, c0 + m0 : c0 + m0 + mw],
                    start=True,
                    stop=True,
                )
            dst = o_t[:, c0 : c0 + gw]
            # alternate wide PSUM->SBUF cast-copies between Vector and Scalar
            # (the only two engines with a PSUM read port; all 128 rows so
            # the stores keep full-partition-width tiles)
            if g % 2 == 0:
                nc.vector.tensor_copy(dst, ps[:, :gw])
            else:
                nc.scalar.activation(dst, ps[:, :gw], mybir.ActivationFunctionType.Copy)
            c0 += gw
            if c0 >= OUT_EDGES[oi + 1]:
                lo, hi = OUT_EDGES[oi], OUT_EDGES[oi + 1]
                out_engines[oi].dma_start(X[:, lo:hi], o_t[:, lo:hi])
                oi += 1
    nc.compile()
    return nc


def _host_fixup(x, C, a, b, c, C_surf, C_bulk):
    """Exact fp32 reference recurrences for the first/last FIX points."""
    n = x.shape[0]
    # left end: exact forward elimination from the Dirichlet row 0
    d0 = C[: FIX + 1].astype(np.float32).copy()
    d0[0] = C_surf
    cp = np.empty(FIX + 1, dtype=np.float32)
    dp = np.empty(FIX + 1, dtype=np.float32)
    cp[0] = np.float32(0.0)
    dp[0] = np.float32(C_surf)
    for i in range(1, FIX + 1):
        den = np.float32(b - a * cp[i - 1])
        cp[i] = np.float32(c / den)
        dp[i] = np.float32((d0[i] - a * dp[i - 1]) / den)
    xl = np.empty(FIX + 1, dtype=np.float32)
    xl[FIX] = x[FIX]
    for i in range(FIX - 1, -1, -1):
        xl[i] = np.float32(dp[i] - cp[i] * xl[i + 1])
    x[:FIX] = xl[:FIX]

    # right end: converged forward state (warmed up), Dirichlet last row
    cpc = np.float32(0.0)
    for _ in range(200):
        den = np.float32(b - a * cpc)
        cpc = np.float32(c / den)
    den_star = np.float32(b - a * cpc)
    warm = 64
    start = n - FIX - warm
    dp_t = np.empty(FIX + 1, dtype=np.float32)
    st = np.float32(0.0)
    for i in range(start, n - 1):
        st = np.float32((np.float32(C[i]) - a * st) / den_star)
        if i >= n - 1 - FIX:
            dp_t[i - (n - 1 - FIX)] = st
    dp_t[FIX] = np.float32(C_bulk)
    xr = np.empty(FIX + 1, dtype=np.float32)
    xr[FIX] = dp_t[FIX]
    for k in range(FIX - 1, -1, -1):
        xr[k] = np.float32(dp_t[k] - cpc * xr[k + 1])
    x[n - 1 - FIX :] = xr
    return x


def kernel(C, dt, C_surf, C_bulk):
    from concourse.bass_utils import run_bass_kernel_spmd

    global LAST_RESULTS, _NC_CACHE

    C = np.asarray(C, dtype=np.float32).reshape(-1)
    assert C.shape[0] == N
    cs = np.float32(np.asarray(C_surf))
    cb = np.float32(np.asarray(C_bulk))
    r, a, b, c = _coeffs(np.asarray(dt))

    w = _fir_taps(a, b, c)
    W = _weight_mat(w)

    # ---- shard: pad + Dirichlet rows, quantize to bf16, then per-core
    # overlapping windows  R_core[p, f] = d[core*PER_CORE + S*f + p - K]
    d_pad = np.zeros(N + 2 * P, dtype=np.float32)
    d_pad[P : P + N] = C
    d_pad[P] = cs               # Dirichlet row 0:    d[0]   -> C_surf
    d_pad[P + N - 1] = cb       # Dirichlet row N-1:  d[N-1] -> C_bulk
    d16 = d_pad.astype(BF16)

    in_maps = []
    for cidx in range(NCORES):
        base = P + cidx * PER_CORE - K
        Rv = np.lib.stride_tricks.as_strided(
            d16[base:], shape=(NCOLS, P), strides=(S * 2, 2)
        )
        in_maps.append({"r_in": np.ascontiguousarray(Rv.T), "w_in": W})

    if _NC_CACHE is None:
        _NC_CACHE = _build_device_program()
    res = run_bass_kernel_spmd(_NC_CACHE, in_maps, core_ids=list(range(NCORES)))
    LAST_RESULTS = res

    # ---- gather: x[S*f + i] = out[i, f] for i < S (rows S..127 are garbage)
    x = np.empty(N, dtype=np.float32)
    for cidx in range(NCORES):
        out = res.results[cidx]["x_out"]  # (128, NCOLS) bf16
        x[cidx * PER_CORE : (cidx + 1) * PER_CORE] = (
            np.ascontiguousarray(out.T[:, :S]).reshape(-1)[:PER_CORE].astype(np.float32)
        )

    return _host_fixup(x, C, a, b, c, cs, cb)
